# revision 8
# baseline (speedup 1.0000x reference)
"""Trainium2 Bass kernel for nn_Deformer (TPS deformer).

Structure:
  host (numpy, f64): bilinear warps of delta/mask/identity-grid, 403x403 TPS
    solves (6 frames), and folding of every term that is affine in the
    per-pixel data into precomputed per-pixel constants.
  device (8 NeuronCores, SPMD): the heavy part -- for every (frame, pixel,
    ctrl-point) triple evaluate the TPS radial kernel and contract against
    6 weight vectors to form the analytic Jacobian of the TPS map, then
    compose the final deformation field.

Device math (validated vs reference to ~9e-6 absmax):
  r2[k,p]  = -2 sx qx - 2 sy qy + |s|^2 + |q|^2          (PE matmul, K=4)
  L[k,p]   = ln(r2 + eps + margin)                        (ACT Ln)
  S[p,j]   = sum_k L[k,p] * w6[k,j], j in 6 contractions  (PE matmul, PSUM acc)
  deform_c = S0c*(E*m) - Sxc*(dx*m) - Syc*(dy*m) + C_c    (DVE)
where C_c absorbs the affine part, the "+1" of d/dq[0.5 r2 ln r2], the
regular grid, and the mask composition -- all host-folded.

Engine-sync discipline: every Trainium instruction here has a single
semaphore-wait slot, so each DRAM input is read by exactly one engine and
each engine "absorbs" a DMA completion via a tiny touch op before real
consumers run (so every real instruction needs at most one new wait).

Sharding: every core processes all 6 frames x a 4608-pixel chunk.
"""
import os
import numpy as np
from contextlib import ExitStack

H = 192; W = 192; NFR = 6
KG = 20; K = KG * KG
LAMBD = 100.0; BIG = 1e8; EPS = 1e-9
DX = 2.0 / (W - 1); DY = 2.0 / (H - 1)

NCORES = 8
HW = H * W
CHUNK = HW // NCORES          # 4608 pixels per core per frame
KT = 100; NCT = 4             # ctrl points split into 4 tiles of 100
BB = 512; NBB = CHUNK // BB   # big blocks (pixels streamed per matmul)
NSB = CHUNK // 128            # 36 sub-blocks of 128 pixels
LN_BIAS = float(EPS + 4e-6)   # margin keeps PSUM-rounded r2+bias > 0
QSF = CHUNK + NCT * KT        # qf columns then sf columns


# ---------------------------------------------------------------- host glue
def _meshgrid_xy(h, w, dtype=np.float64):
    x = np.linspace(-1.0, 1.0, w, dtype=dtype)
    y = np.linspace(-1.0, 1.0, h, dtype=dtype)
    gx, gy = np.meshgrid(x, y)
    return np.stack([gx, gy], 0)[None]


def _backward_warp(img, grid):
    n, c, h, w = img.shape
    x = (grid[..., 0] + 1.0) * 0.5 * (w - 1)
    y = (grid[..., 1] + 1.0) * 0.5 * (h - 1)
    x0 = np.floor(x); y0 = np.floor(y)
    bidx = np.arange(n)[:, None, None]

    def gather(xi, yi):
        valid = (xi >= 0) & (xi <= w - 1) & (yi >= 0) & (yi <= h - 1)
        xc = np.clip(xi, 0, w - 1).astype(np.int64)
        yc = np.clip(yi, 0, h - 1).astype(np.int64)
        v = img[bidx, :, yc, xc]
        return v * valid[..., None].astype(img.dtype)

    wx1 = (x - x0)[..., None]; wy1 = (y - y0)[..., None]
    out = (gather(x0, y0) * (1 - wx1) * (1 - wy1)
           + gather(x0 + 1, y0) * wx1 * (1 - wy1)
           + gather(x0, y0 + 1) * (1 - wx1) * wy1
           + gather(x0 + 1, y0 + 1) * wx1 * wy1)
    return out.transpose(0, 3, 1, 2)


def _tps_U(r2):
    return 0.5 * r2 * np.log(r2 + EPS)


def _host_glue(deform_uv, uv):
    dtype = np.float64
    deform_uv = deform_uv.astype(dtype); uv = uv.astype(dtype)
    n = uv.shape[0]
    xy = _meshgrid_xy(H, W, dtype)
    valid = ((np.abs(deform_uv[..., 0]) <= 1) & (np.abs(deform_uv[..., 1]) <= 1)
             ).astype(dtype)[:, None]
    delta = (deform_uv.transpose(0, 3, 1, 2) - xy) * valid
    delta_ = _backward_warp(np.broadcast_to(delta, (n, 2, H, W)), uv)
    coord_ = _backward_warp(np.broadcast_to(xy, (n, 2, H, W)), uv)
    mask_ = (_backward_warp(np.broadcast_to(valid, (n, 1, H, W)), uv) > 0.5
             ).astype(dtype)

    ii = np.round(np.linspace(0, H - 1, KG)).astype(np.int64)
    jj = np.round(np.linspace(0, W - 1, KG)).astype(np.int64)
    ctrl = (ii[:, None] * W + jj[None, :]).reshape(-1)
    dst = xy[0].reshape(2, -1).T[ctrl]

    wgt = np.zeros((n, K, 2), dtype); aff = np.zeros((n, 3, 2), dtype)
    src_all = np.zeros((n, K, 2), dtype)
    for i in range(n):
        src = coord_[i].reshape(2, -1).T[ctrl]
        m = mask_[i].reshape(-1)[ctrl]
        d2 = ((src[:, None, :] - src[None, :, :]) ** 2).sum(-1)
        Kmat = _tps_U(d2) + np.diag(LAMBD + BIG * (1.0 - m))
        P = np.concatenate([np.ones((K, 1), dtype), src], 1)
        A = np.concatenate([
            np.concatenate([Kmat, P], 1),
            np.concatenate([P.T, np.zeros((3, 3), dtype)], 1)], 0)
        rhs = np.concatenate([dst, np.zeros((3, 2), dtype)], 0)
        sol = np.linalg.solve(A, rhs)
        wgt[i] = sol[:K]; aff[i] = sol[K:]
        src_all[i] = src
    return delta_, coord_, mask_, wgt, aff, src_all, xy


def _prepare_device_inputs(deform_uv, uv):
    delta_, coord_, mask_, wgt, aff, src, xy = _host_glue(deform_uv, uv)
    n = NFR
    qx = coord_[:, 0].reshape(n, -1); qy = coord_[:, 1].reshape(n, -1)
    dlx = delta_[:, 0].reshape(n, -1); dly = delta_[:, 1].reshape(n, -1)
    m = mask_[:, 0].reshape(n, -1)
    gx = xy[0, 0].reshape(-1); gy = xy[0, 1].reshape(-1)

    E = qx * dlx + qy * dly
    W0 = wgt.sum(1)
    Wx = (wgt * src[:, :, 0:1]).sum(1)
    Wy = (wgt * src[:, :, 1:2]).sum(1)
    Cx = (W0[:, 0:1] * E - Wx[:, 0:1] * dlx - Wy[:, 0:1] * dly
          + aff[:, 1, 0:1] * dlx + aff[:, 2, 0:1] * dly)
    Cy = (W0[:, 1:2] * E - Wx[:, 1:2] * dlx - Wy[:, 1:2] * dly
          + aff[:, 1, 1:2] * dlx + aff[:, 2, 1:2] * dly)
    CCx = m * (Cx + gx[None] + 2.0) - 2.0
    CCy = m * (Cy + gy[None] + 2.0) - 2.0
    Em = E * m; Dxm = dlx * m; Dym = dly * m

    # frame-constant device tensors
    sx = src[:, :, 0]; sy = src[:, :, 1]
    sf = np.stack([-2.0 * sx, -2.0 * sy, sx * sx + sy * sy,
                   np.ones_like(sx)], 1).astype(np.float32)        # [n,4,K]
    w6_full = np.stack([wgt[:, :, 0], wgt[:, :, 1],
                        wgt[:, :, 0] * sx, wgt[:, :, 1] * sx,
                        wgt[:, :, 0] * sy, wgt[:, :, 1] * sy], -1)  # [n,K,6]
    w6 = np.zeros((n, KT, NCT * 6), np.float32)
    for c in range(NCT):
        w6[:, :, c * 6:(c + 1) * 6] = w6_full[:, c * KT:(c + 1) * KT, :]
    actc = np.full((128, 1), LN_BIAS, np.float32)

    def chunk_planes(vals, core):
        # vals: list of [n, HW] arrays -> [n, 128, len*NSB] device layout
        sl = slice(core * CHUNK, (core + 1) * CHUNK)
        out = np.zeros((n, 128, len(vals) * NSB), np.float32)
        for j, v in enumerate(vals):
            out[:, :, j * NSB:(j + 1) * NSB] = (
                v[:, sl].reshape(n, NSB, 128).transpose(0, 2, 1))
        return out

    in_maps = []
    for core in range(NCORES):
        sl = slice(core * CHUNK, (core + 1) * CHUNK)
        q2 = qx[:, sl] ** 2 + qy[:, sl] ** 2
        qf = np.stack([qx[:, sl], qy[:, sl], np.ones((n, CHUNK)), q2],
                      1).astype(np.float32)                          # [n,4,CHUNK]
        qsf = np.concatenate([qf, sf], axis=2)                       # [n,4,QSF]
        pp = chunk_planes([Em, Dxm, Dym, CCx, CCy], core)
        in_maps.append({"qsf": np.ascontiguousarray(qsf),
                        "w6": w6, "actc": actc,
                        "pp": np.ascontiguousarray(pp)})
    return in_maps, mask_


# ------------------------------------------------------------- device kernel
_PROGRAM = None


def _build_program():
    import concourse.bass as bass
    import concourse.tile as tile
    from concourse import bacc, mybir
    from concourse.bass import _add_dep_helper

    F32 = mybir.dt.float32
    nc = bacc.Bacc(None, target_bir_lowering=False, debug=False)
    qsf_d = nc.declare_dram_parameter("qsf", [NFR, 4, QSF], F32, isOutput=False)
    w6_d = nc.declare_dram_parameter("w6", [NFR, KT, NCT * 6], F32, isOutput=False)
    pp_d = nc.declare_dram_parameter("pp", [NFR, 128, 5 * NSB], F32, isOutput=False)
    actc_d = nc.declare_dram_parameter("actc", [128, 1], F32, isOutput=False)
    out_d = nc.declare_dram_parameter("out", [NFR, 128, 2 * NSB], F32, isOutput=True)

    def dep(after, before):
        _add_dep_helper(after.ins, before.ins, sync=False,
                        reason="engine touch ordering")

    with ExitStack() as ctx:
        tc = ctx.enter_context(tile.TileContext(nc))
        const_pool = ctx.enter_context(tc.tile_pool(name="const", bufs=1))
        scr_pool = ctx.enter_context(tc.tile_pool(name="scr", bufs=1))
        io_pool = ctx.enter_context(tc.tile_pool(name="io", bufs=2))
        out_pool = ctx.enter_context(tc.tile_pool(name="out", bufs=2))
        L_pool = ctx.enter_context(tc.tile_pool(name="L", bufs=6))
        tmp_pool = ctx.enter_context(tc.tile_pool(name="tmp", bufs=6))
        r2_pool = ctx.enter_context(
            tc.tile_pool(name="r2", bufs=5, space="PSUM"))
        P_pool = ctx.enter_context(
            tc.tile_pool(name="P", bufs=2, space="PSUM"))
        scrp_pool = ctx.enter_context(
            tc.tile_pool(name="scrp", bufs=1, space="PSUM"))

        bias_sb = const_pool.tile([128, 1], F32, tag="bias")
        nc.sync.dma_start(bias_sb[:], actc_d[:])
        scr_act = scr_pool.tile([1, 1], F32, tag="scr_act")
        act_touch = nc.scalar.copy(scr_act[:], bias_sb[0:1, :])
        scr_dve = scr_pool.tile([1, 1], F32, tag="scr_dve")
        scr_ps = scrp_pool.tile([1, 1], F32, tag="scr_ps")

        first_ln = None
        for f in range(NFR):
            qsf_sb = io_pool.tile([4, QSF], F32, tag="qsf")
            nc.sync.dma_start(qsf_sb[:], qsf_d[f])
            w6_sb = io_pool.tile([KT, NCT * 6], F32, tag="w6")
            nc.sync.dma_start(w6_sb[:], w6_d[f])
            pp_sb = io_pool.tile([128, 5 * NSB], F32, tag="pp")
            nc.sync.dma_start(pp_sb[:], pp_d[f])

            # engine touches: absorb one DMA wait each
            d1 = nc.tensor.matmul(scr_ps[:], qsf_sb[0:1, 0:1],
                                  qsf_sb[0:1, 0:1], start=True, stop=True)
            d2 = nc.tensor.matmul(scr_ps[:], w6_sb[0:1, 0:1],
                                  w6_sb[0:1, 0:1], start=True, stop=True)
            dep(d2, d1)
            dv = nc.vector.tensor_copy(scr_dve[:], pp_sb[0:1, 0:1])

            Pbig = P_pool.tile([128, 6 * NSB], F32, tag="P")
            d3 = nc.tensor.matmul(Pbig[0:1, 0:1], qsf_sb[0:1, 0:1],
                                  qsf_sb[0:1, 0:1], start=True, stop=True)
            dep(d3, d2)

            first_mm = None
            first_acc = None
            for b in range(NBB):
                Ls = []
                for c in range(NCT):
                    r2 = r2_pool.tile([KT, BB], F32, tag="r2")
                    mm = nc.tensor.matmul(
                        r2[:], qsf_sb[:, CHUNK + c * KT:CHUNK + (c + 1) * KT],
                        qsf_sb[:, b * BB:(b + 1) * BB], start=True, stop=True)
                    if first_mm is None:
                        first_mm = mm
                        dep(mm, d1)
                    L = L_pool.tile([KT, BB], F32, tag="L")
                    ln = nc.scalar.activation(
                        L[:], r2[:], mybir.ActivationFunctionType.Ln,
                        bias=bias_sb[:KT, :])
                    if first_ln is None:
                        first_ln = ln
                        dep(ln, act_touch)
                    Ls.append(L)
                for c in range(NCT):
                    for s in range(4):
                        g = b * 4 + s
                        acc = nc.tensor.matmul(
                            Pbig[:, g * 6:(g + 1) * 6],
                            Ls[c][:, s * 128:(s + 1) * 128],
                            w6_sb[:, c * 6:(c + 1) * 6],
                            start=(c == 0), stop=(c == NCT - 1))
                        if first_acc is None:
                            first_acc = acc
                            dep(acc, d3)

            # phase 2: deform_c = S0c*Em - Sxc*Dxm - Syc*Dym + C_c
            ot = out_pool.tile([128, 2 * NSB], F32, tag="ot")
            Em_ap = pp_sb[:, 0 * NSB:1 * NSB]
            Dx_ap = pp_sb[:, 1 * NSB:2 * NSB]
            Dy_ap = pp_sb[:, 2 * NSB:3 * NSB]
            first_p2 = None
            for ch in range(2):
                S0 = Pbig[:, 0 + ch::6]
                Sx = Pbig[:, 2 + ch::6]
                Sy = Pbig[:, 4 + ch::6]
                C_ap = pp_sb[:, (3 + ch) * NSB:(4 + ch) * NSB]
                t1 = tmp_pool.tile([128, NSB], F32, tag="t1")
                p2 = nc.vector.tensor_mul(t1[:], S0, Em_ap)
                if first_p2 is None:
                    first_p2 = p2
                    dep(p2, dv)
                t2 = tmp_pool.tile([128, NSB], F32, tag="t2")
                nc.vector.tensor_mul(t2[:], Sx, Dx_ap)
                t3 = tmp_pool.tile([128, NSB], F32, tag="t3")
                nc.vector.tensor_mul(t3[:], Sy, Dy_ap)
                t4 = tmp_pool.tile([128, NSB], F32, tag="t4")
                nc.vector.tensor_sub(t4[:], t1[:], t2[:])
                t5 = tmp_pool.tile([128, NSB], F32, tag="t5")
                nc.vector.tensor_sub(t5[:], t4[:], t3[:])
                nc.vector.tensor_add(ot[:, ch::2], t5[:], C_ap)
            nc.sync.dma_start(out_d[f], ot[:])
    nc.compile()
    return nc


def _run_device(in_maps):
    global _PROGRAM
    from concourse.bass_utils import run_bass_kernel_spmd
    if _PROGRAM is None:
        _PROGRAM = _build_program()
    res = run_bass_kernel_spmd(_PROGRAM, in_maps, core_ids=list(range(NCORES)))
    return [r["out"] for r in res.results]


def _run_sim(in_maps):
    """Numpy simulation of the device program (layout/debug aid)."""
    outs = []
    for im in in_maps:
        qsf, w6, pp = im["qsf"], im["w6"], im["pp"]
        out = np.zeros((NFR, 128, 2 * NSB), np.float32)
        for f in range(NFR):
            P = np.zeros((128, 6 * NSB), np.float32)
            for b in range(NBB):
                for c in range(NCT):
                    r2 = (qsf[f, :, CHUNK + c * KT:CHUNK + (c + 1) * KT].T @
                          qsf[f, :, b * BB:(b + 1) * BB]).astype(np.float32)
                    L = np.log(r2 + np.float32(LN_BIAS)).astype(np.float32)
                    for s in range(4):
                        g = b * 4 + s
                        P[:, g * 6:(g + 1) * 6] += (
                            L[:, s * 128:(s + 1) * 128].T @
                            w6[f, :, c * 6:(c + 1) * 6])
            Em = pp[f, :, 0:NSB]; Dx = pp[f, :, NSB:2 * NSB]
            Dy = pp[f, :, 2 * NSB:3 * NSB]
            for ch in range(2):
                S0 = P[:, 0 + ch::6]; Sx = P[:, 2 + ch::6]; Sy = P[:, 4 + ch::6]
                C = pp[f, :, (3 + ch) * NSB:(4 + ch) * NSB]
                out[f, :, ch::2] = S0 * Em - Sx * Dx - Sy * Dy + C
        outs.append(out)
    return outs


def kernel(deform_uv: np.ndarray, uv: np.ndarray):
    deform_uv = np.asarray(deform_uv); uv = np.asarray(uv)
    in_maps, mask_ = _prepare_device_inputs(deform_uv, uv)
    if os.environ.get("DEFORMER_SIM"):
        core_outs = _run_sim(in_maps)
    else:
        core_outs = _run_device(in_maps)

    deform = np.zeros((NFR, HW, 2), np.float32)
    for core in range(NCORES):
        o = core_outs[core]                      # [NFR, 128, 72]
        sl = slice(core * CHUNK, (core + 1) * CHUNK)
        deform[:, sl] = (o.reshape(NFR, 128, NSB, 2)
                         .transpose(0, 2, 1, 3).reshape(NFR, CHUNK, 2))
    deform = deform.reshape(NFR, H, W, 2)
    return deform, mask_.astype(np.float32)


# revision 15
# speedup vs baseline: 4.4918x; 4.4918x over previous
"""Trainium2 Bass kernel for nn_Deformer (TPS deformer).

Structure:
  host (numpy, f64): bilinear warps of delta/mask/identity-grid, 403x403 TPS
    solves (6 frames), and folding of every term that is affine in the
    per-pixel data into precomputed per-pixel constants.
  device (8 NeuronCores, SPMD): the heavy part -- for every (frame, pixel,
    ctrl-point) triple evaluate the TPS radial kernel and contract against
    6 weight vectors to form the analytic Jacobian of the TPS map, then
    compose the final deformation field.

Device math (validated vs reference to ~9e-6 absmax):
  r2[k,p]  = -2 sx qx - 2 sy qy + |s|^2 + |q|^2          (PE matmul, K=4)
  L[k,p]   = ln(r2 + eps + margin)                        (ACT Ln)
  S[p,j]   = sum_k L[k,p] * w6[k,j], j in 6 contractions  (PE matmul, PSUM acc)
  deform_c = S0c*(E*m) - Sxc*(dx*m) - Syc*(dy*m) + C_c    (DVE)
where C_c absorbs the affine part, the "+1" of d/dq[0.5 r2 ln r2], the
regular grid, and the mask composition -- all host-folded.

Engine-sync discipline: every Trainium instruction here has a single
semaphore-wait slot, so each DRAM input is read by exactly one engine and
each engine "absorbs" a DMA completion via a tiny touch op before real
consumers run (so every real instruction needs at most one new wait).

Sharding: every core processes all 6 frames x a 4608-pixel chunk.
"""
import os
import numpy as np
from contextlib import ExitStack

H = 192; W = 192; NFR = 6
KG = 20; K = KG * KG
LAMBD = 100.0; BIG = 1e8; EPS = 1e-9
DX = 2.0 / (W - 1); DY = 2.0 / (H - 1)

NCORES = 8
HW = H * W
CHUNK = HW // NCORES          # 4608 pixels per core per frame
KT = 128; NCT = 4             # ctrl padded 400->512, 4 tiles of 128
KP = KT * NCT                 # padded ctrl count
BB = 512; NBB = CHUNK // BB   # big blocks (pixels streamed per matmul)
NSB = CHUNK // 128            # 36 sub-blocks of 128 pixels
LN_BIAS = 1e-4                # margin covers bf16 hi/lo r2 residuals (~1e-5)
QSF = CHUNK + KP              # qf columns then sf columns


# ---------------------------------------------------------------- host glue
def _meshgrid_xy(h, w, dtype=np.float64):
    x = np.linspace(-1.0, 1.0, w, dtype=dtype)
    y = np.linspace(-1.0, 1.0, h, dtype=dtype)
    gx, gy = np.meshgrid(x, y)
    return np.stack([gx, gy], 0)[None]


def _backward_warp(img, grid):
    n, c, h, w = img.shape
    x = (grid[..., 0] + 1.0) * 0.5 * (w - 1)
    y = (grid[..., 1] + 1.0) * 0.5 * (h - 1)
    x0 = np.floor(x); y0 = np.floor(y)
    bidx = np.arange(n)[:, None, None]

    def gather(xi, yi):
        valid = (xi >= 0) & (xi <= w - 1) & (yi >= 0) & (yi <= h - 1)
        xc = np.clip(xi, 0, w - 1).astype(np.int64)
        yc = np.clip(yi, 0, h - 1).astype(np.int64)
        v = img[bidx, :, yc, xc]
        return v * valid[..., None].astype(img.dtype)

    wx1 = (x - x0)[..., None]; wy1 = (y - y0)[..., None]
    out = (gather(x0, y0) * (1 - wx1) * (1 - wy1)
           + gather(x0 + 1, y0) * wx1 * (1 - wy1)
           + gather(x0, y0 + 1) * (1 - wx1) * wy1
           + gather(x0 + 1, y0 + 1) * wx1 * wy1)
    return out.transpose(0, 3, 1, 2)


def _tps_U(r2):
    return 0.5 * r2 * np.log(r2 + EPS)


def _host_glue(deform_uv, uv):
    dtype = np.float64
    deform_uv = deform_uv.astype(dtype); uv = uv.astype(dtype)
    n = uv.shape[0]
    xy = _meshgrid_xy(H, W, dtype)
    valid = ((np.abs(deform_uv[..., 0]) <= 1) & (np.abs(deform_uv[..., 1]) <= 1)
             ).astype(dtype)[:, None]
    delta = (deform_uv.transpose(0, 3, 1, 2) - xy) * valid
    delta_ = _backward_warp(np.broadcast_to(delta, (n, 2, H, W)), uv)
    coord_ = _backward_warp(np.broadcast_to(xy, (n, 2, H, W)), uv)
    mask_ = (_backward_warp(np.broadcast_to(valid, (n, 1, H, W)), uv) > 0.5
             ).astype(dtype)

    ii = np.round(np.linspace(0, H - 1, KG)).astype(np.int64)
    jj = np.round(np.linspace(0, W - 1, KG)).astype(np.int64)
    ctrl = (ii[:, None] * W + jj[None, :]).reshape(-1)
    dst = xy[0].reshape(2, -1).T[ctrl]

    wgt = np.zeros((n, K, 2), dtype); aff = np.zeros((n, 3, 2), dtype)
    src_all = np.zeros((n, K, 2), dtype)
    for i in range(n):
        src = coord_[i].reshape(2, -1).T[ctrl]
        m = mask_[i].reshape(-1)[ctrl]
        d2 = ((src[:, None, :] - src[None, :, :]) ** 2).sum(-1)
        Kmat = _tps_U(d2) + np.diag(LAMBD + BIG * (1.0 - m))
        P = np.concatenate([np.ones((K, 1), dtype), src], 1)
        A = np.concatenate([
            np.concatenate([Kmat, P], 1),
            np.concatenate([P.T, np.zeros((3, 3), dtype)], 1)], 0)
        rhs = np.concatenate([dst, np.zeros((3, 2), dtype)], 0)
        sol = np.linalg.solve(A, rhs)
        wgt[i] = sol[:K]; aff[i] = sol[K:]
        src_all[i] = src
    return delta_, coord_, mask_, wgt, aff, src_all, xy


def _prepare_device_inputs(deform_uv, uv):
    delta_, coord_, mask_, wgt, aff, src, xy = _host_glue(deform_uv, uv)
    n = NFR
    qx = coord_[:, 0].reshape(n, -1); qy = coord_[:, 1].reshape(n, -1)
    dlx = delta_[:, 0].reshape(n, -1); dly = delta_[:, 1].reshape(n, -1)
    m = mask_[:, 0].reshape(n, -1)
    gx = xy[0, 0].reshape(-1); gy = xy[0, 1].reshape(-1)

    E = qx * dlx + qy * dly
    W0 = wgt.sum(1)
    Wx = (wgt * src[:, :, 0:1]).sum(1)
    Wy = (wgt * src[:, :, 1:2]).sum(1)
    Cx = (W0[:, 0:1] * E - Wx[:, 0:1] * dlx - Wy[:, 0:1] * dly
          + aff[:, 1, 0:1] * dlx + aff[:, 2, 0:1] * dly)
    Cy = (W0[:, 1:2] * E - Wx[:, 1:2] * dlx - Wy[:, 1:2] * dly
          + aff[:, 1, 1:2] * dlx + aff[:, 2, 1:2] * dly)
    CCx = m * (Cx + gx[None] + 2.0) - 2.0
    CCy = m * (Cy + gy[None] + 2.0) - 2.0
    Em = E * m; Dxm = dlx * m; Dym = dly * m

    # frame-constant device tensors; ctrl padded K -> KP with inert points
    # (sfeat zeros except const rows -> r2 = |q|^2 >= 0, w6 rows 0 -> inert).
    # Coordinates are quantized to bf16 FIRST and |q|^2,|s|^2 shipped as
    # bf16 hi+lo pairs so the K=6 r2 contraction is exact to ~1e-5; the
    # Ln bias margin (1e-4) then guarantees positivity with no clamp pass.
    import ml_dtypes
    bf16 = ml_dtypes.bfloat16

    def q(x):
        return np.asarray(x, np.float32).astype(bf16).astype(np.float32)

    sxq = q(src[:, :, 0]); syq = q(src[:, :, 1])
    s2 = (sxq * sxq + syq * syq).astype(np.float32)
    s2hi = q(s2); s2lo = q(s2 - s2hi)
    sf = np.zeros((n, 6, KP), np.float32)
    sf[:, 0, :K] = -2.0 * sxq; sf[:, 1, :K] = -2.0 * syq
    sf[:, 2, :K] = s2hi; sf[:, 3, :K] = s2lo
    sf[:, 4, :] = 1.0; sf[:, 5, :] = 1.0
    w6_full = np.zeros((n, KP, 6), np.float32)
    w6_full[:, :K] = np.stack(
        [wgt[:, :, 0], wgt[:, :, 1],
         wgt[:, :, 0] * src[:, :, 0], wgt[:, :, 1] * src[:, :, 0],
         wgt[:, :, 0] * src[:, :, 1], wgt[:, :, 1] * src[:, :, 1]], -1)
    w6 = np.zeros((n, KT, NCT * 6), bf16)
    for c in range(NCT):
        w6[:, :, c * 6:(c + 1) * 6] = w6_full[:, c * KT:(c + 1) * KT, :]
    actc = np.full((128, 1), LN_BIAS, np.float32)

    def chunk_planes(vals, core):
        # vals: list of [n, HW] arrays -> [n, 128, len*NSB] device layout
        sl = slice(core * CHUNK, (core + 1) * CHUNK)
        out = np.zeros((n, 128, len(vals) * NSB), np.float32)
        for j, v in enumerate(vals):
            out[:, :, j * NSB:(j + 1) * NSB] = (
                v[:, sl].reshape(n, NSB, 128).transpose(0, 2, 1))
        return out

    in_maps = []
    for core in range(NCORES):
        sl = slice(core * CHUNK, (core + 1) * CHUNK)
        qxq = q(qx[:, sl]); qyq = q(qy[:, sl])
        q2 = (qxq * qxq + qyq * qyq).astype(np.float32)
        q2hi = q(q2); q2lo = q(q2 - q2hi)
        qf = np.stack([qxq, qyq, np.ones((n, CHUNK), np.float32),
                       np.ones((n, CHUNK), np.float32), q2hi, q2lo],
                      1)                                             # [n,6,CHUNK]
        qsf = np.concatenate([qf, sf], axis=2).astype(bf16)          # [n,6,QSF]
        pp = chunk_planes([Em, Dxm, Dym, CCx, CCy], core)
        in_maps.append({"qsf": np.ascontiguousarray(qsf),
                        "w6": w6, "actc": actc,
                        "pp": np.ascontiguousarray(pp)})
    return in_maps, mask_


# ------------------------------------------------------------- device kernel
_PROGRAM = None


def _build_program():
    import concourse.bass as bass
    import concourse.tile as tile
    from concourse import bacc, mybir
    from concourse.bass import _add_dep_helper

    F32 = mybir.dt.float32
    BF16 = mybir.dt.bfloat16
    nc = bacc.Bacc(None, target_bir_lowering=False, debug=False)
    qsf_d = nc.declare_dram_parameter("qsf", [NFR, 6, QSF], BF16, isOutput=False)
    w6_d = nc.declare_dram_parameter("w6", [NFR, KT, NCT * 6], BF16, isOutput=False)
    pp_d = nc.declare_dram_parameter("pp", [NFR, 128, 5 * NSB], F32, isOutput=False)
    actc_d = nc.declare_dram_parameter("actc", [128, 1], F32, isOutput=False)
    out_d = nc.declare_dram_parameter("out", [NFR, 128, 2 * NSB], F32, isOutput=True)

    def dep(after, before):
        _add_dep_helper(after.ins, before.ins, sync=False,
                        reason="engine touch ordering")

    with ExitStack() as ctx:
        tc = ctx.enter_context(tile.TileContext(nc))
        const_pool = ctx.enter_context(tc.tile_pool(name="const", bufs=1))
        scr_pool = ctx.enter_context(tc.tile_pool(name="scr", bufs=1))
        io_pool = ctx.enter_context(tc.tile_pool(name="io", bufs=2))
        out_pool = ctx.enter_context(tc.tile_pool(name="out", bufs=2))
        L_pool = ctx.enter_context(tc.tile_pool(name="L", bufs=6))
        tmp_pool = ctx.enter_context(tc.tile_pool(name="tmp", bufs=6))
        r2_pool = ctx.enter_context(
            tc.tile_pool(name="r2", bufs=5, space="PSUM"))
        P_pool = ctx.enter_context(
            tc.tile_pool(name="P", bufs=2, space="PSUM"))
        scrp_pool = ctx.enter_context(
            tc.tile_pool(name="scrp", bufs=1, space="PSUM"))

        bias_sb = const_pool.tile([128, 1], F32, tag="bias")
        nc.sync.dma_start(bias_sb[:], actc_d[:])
        scr_act = scr_pool.tile([1, 1], F32, tag="scr_act")
        act_touch = nc.scalar.copy(scr_act[:], bias_sb[0:1, :])
        scr_dve = scr_pool.tile([1, 1], F32, tag="scr_dve")
        scr_ps = scrp_pool.tile([1, 1], F32, tag="scr_ps")

        first_ln = None
        for f in range(NFR):
            qsf_sb = io_pool.tile([6, QSF], BF16, tag="qsf")
            nc.sync.dma_start(qsf_sb[:], qsf_d[f])
            w6_sb = io_pool.tile([KT, NCT * 6], BF16, tag="w6")
            nc.sync.dma_start(w6_sb[:], w6_d[f])
            pp_sb = io_pool.tile([128, 5 * NSB], F32, tag="pp")
            nc.sync.dma_start(pp_sb[:], pp_d[f])

            # engine touches: absorb one DMA wait each
            d1 = nc.tensor.matmul(scr_ps[:], qsf_sb[0:1, 0:1],
                                  qsf_sb[0:1, 0:1], start=True, stop=True)
            d2 = nc.tensor.matmul(scr_ps[:], w6_sb[0:1, 0:1],
                                  w6_sb[0:1, 0:1], start=True, stop=True)
            dep(d2, d1)
            dv = nc.vector.tensor_copy(scr_dve[:], pp_sb[0:1, 0:1])

            Pbig = P_pool.tile([128, 6 * NSB], F32, tag="P")
            d3 = nc.tensor.matmul(Pbig[0:1, 0:1], qsf_sb[0:1, 0:1],
                                  qsf_sb[0:1, 0:1], start=True, stop=True)
            dep(d3, d2)

            first_mm = None
            first_acc = None
            for b in range(NBB):
                Ls = []
                for c in range(NCT):
                    r2 = r2_pool.tile([KT, BB], F32, tag="r2")
                    mm = nc.tensor.matmul(
                        r2[:], qsf_sb[:, CHUNK + c * KT:CHUNK + (c + 1) * KT],
                        qsf_sb[:, b * BB:(b + 1) * BB], start=True, stop=True)
                    if first_mm is None:
                        first_mm = mm
                        dep(mm, d1)
                    L = L_pool.tile([KT, BB], BF16, tag="L")
                    ln = nc.scalar.activation(
                        L[:], r2[:], mybir.ActivationFunctionType.Ln,
                        bias=bias_sb[:KT, :])
                    if first_ln is None:
                        first_ln = ln
                        dep(ln, act_touch)
                    Ls.append(L)
                for c in range(NCT):
                    for s in range(4):
                        g = b * 4 + s
                        acc = nc.tensor.matmul(
                            Pbig[:, g * 6:(g + 1) * 6],
                            Ls[c][:, s * 128:(s + 1) * 128],
                            w6_sb[:, c * 6:(c + 1) * 6],
                            start=(c == 0), stop=(c == NCT - 1))
                        if first_acc is None:
                            first_acc = acc
                            dep(acc, d3)

            # phase 2: deform_c = S0c*Em - Sxc*Dxm - Syc*Dym + C_c
            ot = out_pool.tile([128, 2 * NSB], F32, tag="ot")
            Em_ap = pp_sb[:, 0 * NSB:1 * NSB]
            Dx_ap = pp_sb[:, 1 * NSB:2 * NSB]
            Dy_ap = pp_sb[:, 2 * NSB:3 * NSB]
            first_p2 = None
            for ch in range(2):
                S0 = Pbig[:, 0 + ch::6]
                Sx = Pbig[:, 2 + ch::6]
                Sy = Pbig[:, 4 + ch::6]
                C_ap = pp_sb[:, (3 + ch) * NSB:(4 + ch) * NSB]
                t1 = tmp_pool.tile([128, NSB], F32, tag="t1")
                p2 = nc.vector.tensor_mul(t1[:], S0, Em_ap)
                if first_p2 is None:
                    first_p2 = p2
                    dep(p2, dv)
                t2 = tmp_pool.tile([128, NSB], F32, tag="t2")
                nc.vector.tensor_mul(t2[:], Sx, Dx_ap)
                t3 = tmp_pool.tile([128, NSB], F32, tag="t3")
                nc.vector.tensor_mul(t3[:], Sy, Dy_ap)
                t4 = tmp_pool.tile([128, NSB], F32, tag="t4")
                nc.vector.tensor_sub(t4[:], t1[:], t2[:])
                t5 = tmp_pool.tile([128, NSB], F32, tag="t5")
                nc.vector.tensor_sub(t5[:], t4[:], t3[:])
                nc.vector.tensor_add(ot[:, ch::2], t5[:], C_ap)
            nc.sync.dma_start(out_d[f], ot[:])
    nc.compile()
    return nc


def _run_device(in_maps):
    global _PROGRAM
    from concourse.bass_utils import run_bass_kernel_spmd
    if _PROGRAM is None:
        _PROGRAM = _build_program()
    res = run_bass_kernel_spmd(_PROGRAM, in_maps, core_ids=list(range(NCORES)))
    return [r["out"] for r in res.results]


def _run_sim(in_maps):
    """Numpy simulation of the device program (layout/debug aid)."""
    outs = []
    for im in in_maps:
        qsf, w6, pp = im["qsf"], im["w6"], im["pp"]
        out = np.zeros((NFR, 128, 2 * NSB), np.float32)
        for f in range(NFR):
            P = np.zeros((128, 6 * NSB), np.float32)
            for b in range(NBB):
                for c in range(NCT):
                    r2 = (qsf[f, :, CHUNK + c * KT:CHUNK + (c + 1) * KT]
                          .astype(np.float32).T @
                          qsf[f, :, b * BB:(b + 1) * BB].astype(np.float32))
                    r2 = np.maximum(r2, 0.0)
                    L = np.log(r2 + np.float32(LN_BIAS)).astype(np.float32)
                    for s in range(4):
                        g = b * 4 + s
                        P[:, g * 6:(g + 1) * 6] += (
                            L[:, s * 128:(s + 1) * 128].T @
                            w6[f, :, c * 6:(c + 1) * 6].astype(np.float32))
            Em = pp[f, :, 0:NSB]; Dx = pp[f, :, NSB:2 * NSB]
            Dy = pp[f, :, 2 * NSB:3 * NSB]
            for ch in range(2):
                S0 = P[:, 0 + ch::6]; Sx = P[:, 2 + ch::6]; Sy = P[:, 4 + ch::6]
                C = pp[f, :, (3 + ch) * NSB:(4 + ch) * NSB]
                out[f, :, ch::2] = S0 * Em - Sx * Dx - Sy * Dy + C
        outs.append(out)
    return outs


def kernel(deform_uv: np.ndarray, uv: np.ndarray):
    deform_uv = np.asarray(deform_uv); uv = np.asarray(uv)
    in_maps, mask_ = _prepare_device_inputs(deform_uv, uv)
    if os.environ.get("DEFORMER_SIM"):
        core_outs = _run_sim(in_maps)
    else:
        core_outs = _run_device(in_maps)

    deform = np.zeros((NFR, HW, 2), np.float32)
    for core in range(NCORES):
        o = core_outs[core]                      # [NFR, 128, 72]
        sl = slice(core * CHUNK, (core + 1) * CHUNK)
        deform[:, sl] = (o.reshape(NFR, 128, NSB, 2)
                         .transpose(0, 2, 1, 3).reshape(NFR, CHUNK, 2))
    deform = deform.reshape(NFR, H, W, 2)
    return deform, mask_.astype(np.float32)


# revision 17
# speedup vs baseline: 5.0734x; 1.1295x over previous
"""Trainium2 Bass kernel for nn_Deformer (TPS deformer).

Structure:
  host (numpy, f64): bilinear warps of delta/mask/identity-grid, 403x403 TPS
    solves (6 frames), and folding of every term that is affine in the
    per-pixel data into precomputed per-pixel constants.
  device (8 NeuronCores, SPMD): the heavy part -- for every (frame, pixel,
    ctrl-point) triple evaluate the TPS radial kernel and contract against
    6 weight vectors to form the analytic Jacobian of the TPS map, then
    compose the final deformation field.

Device math (validated vs reference to ~9e-6 absmax):
  r2[k,p]  = -2 sx qx - 2 sy qy + |s|^2 + |q|^2          (PE matmul, K=4)
  L[k,p]   = ln(r2 + eps + margin)                        (ACT Ln)
  S[p,j]   = sum_k L[k,p] * w6[k,j], j in 6 contractions  (PE matmul, PSUM acc)
  deform_c = S0c*(E*m) - Sxc*(dx*m) - Syc*(dy*m) + C_c    (DVE)
where C_c absorbs the affine part, the "+1" of d/dq[0.5 r2 ln r2], the
regular grid, and the mask composition -- all host-folded.

Engine-sync discipline: every Trainium instruction here has a single
semaphore-wait slot, so each DRAM input is read by exactly one engine and
each engine "absorbs" a DMA completion via a tiny touch op before real
consumers run (so every real instruction needs at most one new wait).

Sharding: every core processes all 6 frames x a 4608-pixel chunk.
"""
import os
import numpy as np
from contextlib import ExitStack

H = 192; W = 192; NFR = 6
KG = 20; K = KG * KG
LAMBD = 100.0; BIG = 1e8; EPS = 1e-9
DX = 2.0 / (W - 1); DY = 2.0 / (H - 1)

NCORES = 8
HW = H * W
CHUNK = HW // NCORES          # 4608 pixels per core per frame
KT = 128; NCT = 4             # ctrl padded 400->512, 4 tiles of 128
KP = KT * NCT                 # padded ctrl count
BB = 512; NBB = CHUNK // BB   # big blocks (pixels streamed per matmul)
NSB = CHUNK // 128            # 36 sub-blocks of 128 pixels
LN_BIAS = 1e-4                # margin covers bf16 hi/lo r2 residuals (~1e-5)
QSF = CHUNK + KP              # qf columns then sf columns


# ---------------------------------------------------------------- host glue
def _meshgrid_xy(h, w, dtype=np.float64):
    x = np.linspace(-1.0, 1.0, w, dtype=dtype)
    y = np.linspace(-1.0, 1.0, h, dtype=dtype)
    gx, gy = np.meshgrid(x, y)
    return np.stack([gx, gy], 0)[None]


def _backward_warp(img, grid):
    n, c, h, w = img.shape
    x = (grid[..., 0] + 1.0) * 0.5 * (w - 1)
    y = (grid[..., 1] + 1.0) * 0.5 * (h - 1)
    x0 = np.floor(x); y0 = np.floor(y)
    bidx = np.arange(n)[:, None, None]

    def gather(xi, yi):
        valid = (xi >= 0) & (xi <= w - 1) & (yi >= 0) & (yi <= h - 1)
        xc = np.clip(xi, 0, w - 1).astype(np.int64)
        yc = np.clip(yi, 0, h - 1).astype(np.int64)
        v = img[bidx, :, yc, xc]
        return v * valid[..., None].astype(img.dtype)

    wx1 = (x - x0)[..., None]; wy1 = (y - y0)[..., None]
    out = (gather(x0, y0) * (1 - wx1) * (1 - wy1)
           + gather(x0 + 1, y0) * wx1 * (1 - wy1)
           + gather(x0, y0 + 1) * (1 - wx1) * wy1
           + gather(x0 + 1, y0 + 1) * wx1 * wy1)
    return out.transpose(0, 3, 1, 2)


def _tps_U(r2):
    return 0.5 * r2 * np.log(r2 + EPS)


def _host_glue(deform_uv, uv):
    dtype = np.float64
    deform_uv = deform_uv.astype(dtype); uv = uv.astype(dtype)
    n = uv.shape[0]
    xy = _meshgrid_xy(H, W, dtype)
    valid = ((np.abs(deform_uv[..., 0]) <= 1) & (np.abs(deform_uv[..., 1]) <= 1)
             ).astype(dtype)[:, None]
    delta = (deform_uv.transpose(0, 3, 1, 2) - xy) * valid
    delta_ = _backward_warp(np.broadcast_to(delta, (n, 2, H, W)), uv)
    coord_ = _backward_warp(np.broadcast_to(xy, (n, 2, H, W)), uv)
    mask_ = (_backward_warp(np.broadcast_to(valid, (n, 1, H, W)), uv) > 0.5
             ).astype(dtype)

    ii = np.round(np.linspace(0, H - 1, KG)).astype(np.int64)
    jj = np.round(np.linspace(0, W - 1, KG)).astype(np.int64)
    ctrl = (ii[:, None] * W + jj[None, :]).reshape(-1)
    dst = xy[0].reshape(2, -1).T[ctrl]

    wgt = np.zeros((n, K, 2), dtype); aff = np.zeros((n, 3, 2), dtype)
    src_all = np.zeros((n, K, 2), dtype)
    for i in range(n):
        src = coord_[i].reshape(2, -1).T[ctrl]
        m = mask_[i].reshape(-1)[ctrl]
        d2 = ((src[:, None, :] - src[None, :, :]) ** 2).sum(-1)
        Kmat = _tps_U(d2) + np.diag(LAMBD + BIG * (1.0 - m))
        P = np.concatenate([np.ones((K, 1), dtype), src], 1)
        A = np.concatenate([
            np.concatenate([Kmat, P], 1),
            np.concatenate([P.T, np.zeros((3, 3), dtype)], 1)], 0)
        rhs = np.concatenate([dst, np.zeros((3, 2), dtype)], 0)
        sol = np.linalg.solve(A, rhs)
        wgt[i] = sol[:K]; aff[i] = sol[K:]
        src_all[i] = src
    return delta_, coord_, mask_, wgt, aff, src_all, xy


def _prepare_device_inputs(deform_uv, uv):
    delta_, coord_, mask_, wgt, aff, src, xy = _host_glue(deform_uv, uv)
    n = NFR
    qx = coord_[:, 0].reshape(n, -1); qy = coord_[:, 1].reshape(n, -1)
    dlx = delta_[:, 0].reshape(n, -1); dly = delta_[:, 1].reshape(n, -1)
    m = mask_[:, 0].reshape(n, -1)
    gx = xy[0, 0].reshape(-1); gy = xy[0, 1].reshape(-1)

    E = qx * dlx + qy * dly
    W0 = wgt.sum(1)
    Wx = (wgt * src[:, :, 0:1]).sum(1)
    Wy = (wgt * src[:, :, 1:2]).sum(1)
    Cx = (W0[:, 0:1] * E - Wx[:, 0:1] * dlx - Wy[:, 0:1] * dly
          + aff[:, 1, 0:1] * dlx + aff[:, 2, 0:1] * dly)
    Cy = (W0[:, 1:2] * E - Wx[:, 1:2] * dlx - Wy[:, 1:2] * dly
          + aff[:, 1, 1:2] * dlx + aff[:, 2, 1:2] * dly)
    CCx = m * (Cx + gx[None] + 2.0) - 2.0
    CCy = m * (Cy + gy[None] + 2.0) - 2.0
    Em = E * m; Dxm = dlx * m; Dym = dly * m

    # frame-constant device tensors; ctrl padded K -> KP with inert points
    # (sfeat zeros except const rows -> r2 = |q|^2 >= 0, w6 rows 0 -> inert).
    # Coordinates are quantized to bf16 FIRST and |q|^2,|s|^2 shipped as
    # bf16 hi+lo pairs so the K=6 r2 contraction is exact to ~1e-5; the
    # Ln bias margin (1e-4) then guarantees positivity with no clamp pass.
    import ml_dtypes
    bf16 = ml_dtypes.bfloat16

    def q(x):
        return np.asarray(x, np.float32).astype(bf16).astype(np.float32)

    sxq = q(src[:, :, 0]); syq = q(src[:, :, 1])
    s2 = (sxq * sxq + syq * syq).astype(np.float32)
    s2hi = q(s2); s2lo = q(s2 - s2hi)
    sf = np.zeros((n, 6, KP), np.float32)
    sf[:, 0, :K] = -2.0 * sxq; sf[:, 1, :K] = -2.0 * syq
    sf[:, 2, :K] = s2hi; sf[:, 3, :K] = s2lo
    sf[:, 4, :] = 1.0; sf[:, 5, :] = 1.0
    w6_full = np.zeros((n, KP, 6), np.float32)
    w6_full[:, :K] = np.stack(
        [wgt[:, :, 0], wgt[:, :, 1],
         wgt[:, :, 0] * src[:, :, 0], wgt[:, :, 1] * src[:, :, 0],
         wgt[:, :, 0] * src[:, :, 1], wgt[:, :, 1] * src[:, :, 1]], -1)
    w6 = np.zeros((n, KT, NCT * 6), bf16)
    for c in range(NCT):
        w6[:, :, c * 6:(c + 1) * 6] = w6_full[:, c * KT:(c + 1) * KT, :]
    actc = np.full((128, 1), LN_BIAS, np.float32)

    def chunk_planes(vals, core):
        # vals: list of [n, HW] arrays -> [n, 128, len*NSB] device layout
        sl = slice(core * CHUNK, (core + 1) * CHUNK)
        out = np.zeros((n, 128, len(vals) * NSB), np.float32)
        for j, v in enumerate(vals):
            out[:, :, j * NSB:(j + 1) * NSB] = (
                v[:, sl].reshape(n, NSB, 128).transpose(0, 2, 1))
        return out

    in_maps = []
    for core in range(NCORES):
        sl = slice(core * CHUNK, (core + 1) * CHUNK)
        qxq = q(qx[:, sl]); qyq = q(qy[:, sl])
        q2 = (qxq * qxq + qyq * qyq).astype(np.float32)
        q2hi = q(q2); q2lo = q(q2 - q2hi)
        qf = np.stack([qxq, qyq, np.ones((n, CHUNK), np.float32),
                       np.ones((n, CHUNK), np.float32), q2hi, q2lo],
                      1)                                             # [n,6,CHUNK]
        qsf = np.concatenate([qf, sf], axis=2).astype(bf16)          # [n,6,QSF]
        pp = chunk_planes([Em, Dxm, Dym, CCx, CCy], core)
        in_maps.append({"qsf": np.ascontiguousarray(qsf),
                        "w6": w6, "actc": actc,
                        "pp": np.ascontiguousarray(pp)})
    return in_maps, mask_


# ------------------------------------------------------------- device kernel
_PROGRAM = None


def _build_program():
    import concourse.bass as bass
    import concourse.tile as tile
    from concourse import bacc, mybir
    from concourse.bass import _add_dep_helper

    F32 = mybir.dt.float32
    BF16 = mybir.dt.bfloat16
    nc = bacc.Bacc(None, target_bir_lowering=False, debug=False)
    qsf_d = nc.declare_dram_parameter("qsf", [NFR, 6, QSF], BF16, isOutput=False)
    w6_d = nc.declare_dram_parameter("w6", [NFR, KT, NCT * 6], BF16, isOutput=False)
    pp_d = nc.declare_dram_parameter("pp", [NFR, 128, 5 * NSB], F32, isOutput=False)
    actc_d = nc.declare_dram_parameter("actc", [128, 1], F32, isOutput=False)
    out_d = nc.declare_dram_parameter("out", [NFR, 128, 2 * NSB], F32, isOutput=True)

    def dep(after, before):
        _add_dep_helper(after.ins, before.ins, sync=False,
                        reason="engine touch ordering")

    with ExitStack() as ctx:
        tc = ctx.enter_context(tile.TileContext(nc))
        const_pool = ctx.enter_context(tc.tile_pool(name="const", bufs=1))
        scr_pool = ctx.enter_context(tc.tile_pool(name="scr", bufs=1))
        io_pool = ctx.enter_context(tc.tile_pool(name="io", bufs=2))
        out_pool = ctx.enter_context(tc.tile_pool(name="out", bufs=2))
        L_pool = ctx.enter_context(tc.tile_pool(name="L", bufs=6))
        tmp_pool = ctx.enter_context(tc.tile_pool(name="tmp", bufs=6))
        r2_pool = ctx.enter_context(
            tc.tile_pool(name="r2", bufs=3, space="PSUM"))
        P_pool = ctx.enter_context(
            tc.tile_pool(name="P", bufs=1, space="PSUM"))
        scrp_pool = ctx.enter_context(
            tc.tile_pool(name="scrp", bufs=1, space="PSUM"))

        bias_sb = const_pool.tile([128, 1], F32, tag="bias")
        nc.sync.dma_start(bias_sb[:], actc_d[:])
        scr_act = scr_pool.tile([1, 1], F32, tag="scr_act")
        act_touch = nc.scalar.copy(scr_act[:], bias_sb[0:1, :])
        scr_dve = scr_pool.tile([1, 1], F32, tag="scr_dve")
        scr_ps = scrp_pool.tile([1, 1], F32, tag="scr_ps")

        first_ln = None
        for f in range(NFR):
            qsf_sb = io_pool.tile([6, QSF], BF16, tag="qsf")
            nc.sync.dma_start(qsf_sb[:], qsf_d[f])
            w6_sb = io_pool.tile([KT, NCT * 6], BF16, tag="w6")
            nc.sync.dma_start(w6_sb[:], w6_d[f])
            pp_sb = io_pool.tile([128, 5 * NSB], F32, tag="pp")
            nc.sync.dma_start(pp_sb[:], pp_d[f])

            # engine touches: absorb one DMA wait each
            d1 = nc.tensor.matmul(scr_ps[:], qsf_sb[0:1, 0:1],
                                  qsf_sb[0:1, 0:1], start=True, stop=True)
            d2 = nc.tensor.matmul(scr_ps[:], w6_sb[0:1, 0:1],
                                  w6_sb[0:1, 0:1], start=True, stop=True)
            dep(d2, d1)
            dv = nc.vector.tensor_copy(scr_dve[:], pp_sb[0:1, 0:1])

            Pbig = P_pool.tile([128, 6 * NSB], F32, tag="P")
            d3 = nc.tensor.matmul(Pbig[0:1, 0:1], qsf_sb[0:1, 0:1],
                                  qsf_sb[0:1, 0:1], start=True, stop=True)
            dep(d3, d2)

            first_mm = None
            first_acc = None
            for b in range(NBB):
                Ls = []
                for half in range(NCT // 2):
                    # two ctrl-tiles share one 2-bank PSUM tile -> one Ln op
                    r2 = r2_pool.tile([KT, 2 * BB], F32, tag="r2")
                    for cc in range(2):
                        c = 2 * half + cc
                        mm = nc.tensor.matmul(
                            r2[:, cc * BB:(cc + 1) * BB],
                            qsf_sb[:, CHUNK + c * KT:CHUNK + (c + 1) * KT],
                            qsf_sb[:, b * BB:(b + 1) * BB],
                            start=True, stop=True)
                        if first_mm is None:
                            first_mm = mm
                            dep(mm, d1)
                    L = L_pool.tile([KT, 2 * BB], BF16, tag="L")
                    ln = nc.scalar.activation(
                        L[:], r2[:], mybir.ActivationFunctionType.Ln,
                        bias=bias_sb[:KT, :])
                    if first_ln is None:
                        first_ln = ln
                        dep(ln, act_touch)
                    Ls.append(L)
                for c in range(NCT):
                    L = Ls[c // 2]
                    off = (c % 2) * BB
                    for s in range(4):
                        g = b * 4 + s
                        acc = nc.tensor.matmul(
                            Pbig[:, g * 6:(g + 1) * 6],
                            L[:, off + s * 128:off + (s + 1) * 128],
                            w6_sb[:, c * 6:(c + 1) * 6],
                            start=(c == 0), stop=(c == NCT - 1))
                        if first_acc is None:
                            first_acc = acc
                            dep(acc, d3)

            # phase 2: deform_c = S0c*Em - Sxc*Dxm - Syc*Dym + C_c
            ot = out_pool.tile([128, 2 * NSB], F32, tag="ot")
            Em_ap = pp_sb[:, 0 * NSB:1 * NSB]
            Dx_ap = pp_sb[:, 1 * NSB:2 * NSB]
            Dy_ap = pp_sb[:, 2 * NSB:3 * NSB]
            first_p2 = None
            for ch in range(2):
                S0 = Pbig[:, 0 + ch::6]
                Sx = Pbig[:, 2 + ch::6]
                Sy = Pbig[:, 4 + ch::6]
                C_ap = pp_sb[:, (3 + ch) * NSB:(4 + ch) * NSB]
                t1 = tmp_pool.tile([128, NSB], F32, tag="t1")
                p2 = nc.vector.tensor_mul(t1[:], S0, Em_ap)
                if first_p2 is None:
                    first_p2 = p2
                    dep(p2, dv)
                t2 = tmp_pool.tile([128, NSB], F32, tag="t2")
                nc.vector.tensor_mul(t2[:], Sx, Dx_ap)
                t3 = tmp_pool.tile([128, NSB], F32, tag="t3")
                nc.vector.tensor_mul(t3[:], Sy, Dy_ap)
                t4 = tmp_pool.tile([128, NSB], F32, tag="t4")
                nc.vector.tensor_sub(t4[:], t1[:], t2[:])
                t5 = tmp_pool.tile([128, NSB], F32, tag="t5")
                nc.vector.tensor_sub(t5[:], t4[:], t3[:])
                nc.vector.tensor_add(ot[:, ch::2], t5[:], C_ap)
            nc.sync.dma_start(out_d[f], ot[:])
    nc.compile()
    return nc


def _run_device(in_maps):
    global _PROGRAM
    from concourse.bass_utils import run_bass_kernel_spmd
    if _PROGRAM is None:
        _PROGRAM = _build_program()
    res = run_bass_kernel_spmd(_PROGRAM, in_maps, core_ids=list(range(NCORES)))
    return [r["out"] for r in res.results]


def _run_sim(in_maps):
    """Numpy simulation of the device program (layout/debug aid)."""
    outs = []
    for im in in_maps:
        qsf, w6, pp = im["qsf"], im["w6"], im["pp"]
        out = np.zeros((NFR, 128, 2 * NSB), np.float32)
        for f in range(NFR):
            P = np.zeros((128, 6 * NSB), np.float32)
            for b in range(NBB):
                for c in range(NCT):
                    r2 = (qsf[f, :, CHUNK + c * KT:CHUNK + (c + 1) * KT]
                          .astype(np.float32).T @
                          qsf[f, :, b * BB:(b + 1) * BB].astype(np.float32))
                    r2 = np.maximum(r2, 0.0)
                    L = np.log(r2 + np.float32(LN_BIAS)).astype(np.float32)
                    for s in range(4):
                        g = b * 4 + s
                        P[:, g * 6:(g + 1) * 6] += (
                            L[:, s * 128:(s + 1) * 128].T @
                            w6[f, :, c * 6:(c + 1) * 6].astype(np.float32))
            Em = pp[f, :, 0:NSB]; Dx = pp[f, :, NSB:2 * NSB]
            Dy = pp[f, :, 2 * NSB:3 * NSB]
            for ch in range(2):
                S0 = P[:, 0 + ch::6]; Sx = P[:, 2 + ch::6]; Sy = P[:, 4 + ch::6]
                C = pp[f, :, (3 + ch) * NSB:(4 + ch) * NSB]
                out[f, :, ch::2] = S0 * Em - Sx * Dx - Sy * Dy + C
        outs.append(out)
    return outs


def kernel(deform_uv: np.ndarray, uv: np.ndarray):
    deform_uv = np.asarray(deform_uv); uv = np.asarray(uv)
    in_maps, mask_ = _prepare_device_inputs(deform_uv, uv)
    if os.environ.get("DEFORMER_SIM"):
        core_outs = _run_sim(in_maps)
    else:
        core_outs = _run_device(in_maps)

    deform = np.zeros((NFR, HW, 2), np.float32)
    for core in range(NCORES):
        o = core_outs[core]                      # [NFR, 128, 72]
        sl = slice(core * CHUNK, (core + 1) * CHUNK)
        deform[:, sl] = (o.reshape(NFR, 128, NSB, 2)
                         .transpose(0, 2, 1, 3).reshape(NFR, CHUNK, 2))
    deform = deform.reshape(NFR, H, W, 2)
    return deform, mask_.astype(np.float32)
